# revision 15
# baseline (speedup 1.0000x reference)
"""Bidirectional Mamba2 block on 8 Trainium2 NeuronCores.

Sharding: core = (direction, batch, head-half). Each core runs in_proj,
causal depthwise conv, a chunked (SSD) selective scan for its 16 heads,
gated-RMSNorm partials and its half of out_proj, producing a partial
product P [1024,1024] and per-token sum-of-squares ss [1024]. The host
combines halves with rsqrt, then a second tiny SPMD kernel applies the
final concat+linear Wo (token-parallel across all 8 cores).

All matmuls run in bf16 with fp32 PSUM accumulation; transcendentals and
the decay cumsums are fp32.
"""
import numpy as np
import ml_dtypes

import concourse.bass as bass
import concourse.bacc as bacc
import concourse.mybir as mybir
import concourse.tile as tile
from concourse import bass_utils

BF16 = ml_dtypes.bfloat16
F32 = mybir.dt.float32
BF = mybir.dt.bfloat16
AF = mybir.ActivationFunctionType
OP = mybir.AluOpType

L = 1024          # sequence length
DM = 1024         # d_model
HL = 16           # heads per core
Q = 128           # chunk length
NCH = L // Q      # 8 chunks
CONVT = 9         # conv channel tiles: 8 xs + 1 (B|C)
BIG = 30000.0


# ----------------------------------------------------------------------------
# Launch 1: per-(dir, batch, half) Mamba core
# ----------------------------------------------------------------------------
def build_core():
    nc = bacc.Bacc()

    xT_d = nc.dram_tensor("xT", [8, 128, L], BF, kind="ExternalInput")
    wxbc_d = nc.dram_tensor("wxbc", [8, 128, 1152], BF, kind="ExternalInput")
    wz_d = nc.dram_tensor("wz", [8, 128, 1024], BF, kind="ExternalInput")
    wdt_d = nc.dram_tensor("wdt", [8, 128, HL], BF, kind="ExternalInput")
    wout_d = nc.dram_tensor("wout", [8, 128, 1024], BF, kind="ExternalInput")
    convw_d = nc.dram_tensor("convw", [CONVT, 128, 4], F32, kind="ExternalInput")
    convb_d = nc.dram_tensor("convb", [CONVT, 128], F32, kind="ExternalInput")
    dtb_d = nc.dram_tensor("dtb", [HL], F32, kind="ExternalInput")
    negA_d = nc.dram_tensor("negA", [HL], F32, kind="ExternalInput")
    diag_d = nc.dram_tensor("diag", [128, HL, 128], BF, kind="ExternalInput")
    lowbig_d = nc.dram_tensor("lowbig", [128, 128], F32, kind="ExternalInput")
    idf_d = nc.dram_tensor("idf", [128, 128], F32, kind="ExternalInput")
    idb_d = nc.dram_tensor("idb", [128, 128], BF, kind="ExternalInput")

    cum_dram = nc.dram_tensor("cum_dram", [HL, L], F32)
    ecum_dram = nc.dram_tensor("ecum_dram", [HL, L], BF)
    P_d = nc.dram_tensor("P", [8, 128, 1024], F32, kind="ExternalOutput")
    ss_d = nc.dram_tensor("ss", [8, 128], F32, kind="ExternalOutput")

    with tile.TileContext(nc) as tc:
        with (
            tc.tile_pool(name="res", bufs=1) as res,
            tc.tile_pool(name="work", bufs=2) as work,
            tc.tile_pool(name="work1", bufs=1) as work1,
            tc.tile_pool(name="ps_main", bufs=2, space="PSUM") as ps_main,
            tc.tile_pool(name="ps_tr", bufs=2, space="PSUM") as ps_tr,
            tc.tile_pool(name="ps_g", bufs=1, space="PSUM") as ps_g,
            tc.tile_pool(name="ps_y", bufs=1, space="PSUM") as ps_y,
            tc.tile_pool(name="ps_u", bufs=1, space="PSUM") as ps_u,
        ):
            # ---- resident inputs -------------------------------------------
            xT = res.tile([128, 8, L], BF)
            nc.sync.dma_start(xT[:], xT_d.rearrange("k p t -> p k t"))
            wxbc = res.tile([128, 8, 1152], BF)
            nc.sync.dma_start(wxbc[:], wxbc_d.rearrange("k p t -> p k t"))
            wz = res.tile([128, 8, 1024], BF)
            nc.sync.dma_start(wz[:], wz_d.rearrange("k p t -> p k t"))
            wdt = res.tile([128, 8, HL], BF)
            nc.sync.dma_start(wdt[:], wdt_d.rearrange("k p t -> p k t"))
            wout = res.tile([128, 8, 1024], BF)
            nc.sync.dma_start(wout[:], wout_d.rearrange("k p t -> p k t"))
            convw = res.tile([128, CONVT, 4], F32)
            nc.sync.dma_start(convw[:], convw_d.rearrange("k p t -> p k t"))
            convb = res.tile([128, CONVT], F32)
            nc.sync.dma_start(convb[:], convb_d.rearrange("k p -> p k"))
            dtb = res.tile([HL, 1], F32)
            nc.sync.dma_start(dtb[:], dtb_d.rearrange("(h o) -> h o", o=1))
            negA = res.tile([HL, 1], F32)
            nc.sync.dma_start(negA[:], negA_d.rearrange("(h o) -> h o", o=1))
            diag = res.tile([128, HL, 128], BF)
            nc.sync.dma_start(diag[:], diag_d[:, :, :])
            lowbig = res.tile([128, 128], F32)
            nc.sync.dma_start(lowbig[:], lowbig_d[:, :])
            idf = res.tile([128, 128], F32)
            nc.sync.dma_start(idf[:], idf_d[:, :])
            idb = res.tile([128, 128], BF)
            nc.sync.dma_start(idb[:], idb_d[:, :])

            # ---- resident intermediates ------------------------------------
            xbcT = res.tile([128, CONVT, 3 + L], BF, tag="bigT")   # conv input (padded)
            BCt = res.tile([128, L], BF)        # rows 0:64 = B^T, 64:128 = C^T
            zs = res.tile([128, NCH, 1024], BF)  # silu(z), token-major
            xsTok = res.tile([128, NCH, 1024], BF)  # xs, token-major [t, c, hp]
            gb = res.tile([128, NCH, 1024], BF)  # gated y, token-major
            dtT = res.tile([HL, L], F32)
            logdA = res.tile([HL, L], F32)
            cum = res.tile([HL, L], F32)
            cumlog2 = res.tile([HL, L], F32)
            expcum_cm = res.tile([HL, L], BF)
            expcumT = res.tile([128, NCH, HL], F32)
            negclT = res.tile([128, NCH, HL], F32)
            dstateT = res.tile([128, NCH, HL], F32)
            lam = res.tile([64, NCH, HL], F32)
            Snp = res.tile([64, HL, 64], BF)    # states: [n, h, p]
            ss_sb = res.tile([128, NCH], F32)
            ones16 = res.tile([HL, Q], F32)
            nc.vector.memset(ones16[:], 1.0)
            nc.vector.memset(xbcT[:, :, 0:3], 0.0)

            # ---- in_proj ---------------------------------------------------
            # xBC block, column-major: psum[cols, t] = Win_xBC.T @ x.T
            for ct in range(CONVT):
                for t2 in range(2):
                    pt = ps_main.tile([128, 512], F32, tag="mm")
                    for kt in range(8):
                        nc.tensor.matmul(
                            pt[:], wxbc[:, kt, ct * 128:(ct + 1) * 128],
                            xT[:, kt, t2 * 512:(t2 + 1) * 512],
                            start=(kt == 0), stop=(kt == 7))
                    nc.scalar.copy(xbcT[:, ct, 3 + t2 * 512: 3 + (t2 + 1) * 512], pt[:])
            # z block, token-major, silu fused into eviction
            for tt in range(8):
                for zt in range(2):
                    pt = ps_main.tile([128, 512], F32, tag="mm")
                    for kt in range(8):
                        nc.tensor.matmul(
                            pt[:], xT[:, kt, tt * 128:(tt + 1) * 128],
                            wz[:, kt, zt * 512:(zt + 1) * 512],
                            start=(kt == 0), stop=(kt == 7))
                    sigz = work.tile([128, 512], BF, tag="sigz", name="sigz")
                    nc.scalar.activation(sigz[:], pt[:], AF.Sigmoid)
                    nc.vector.tensor_tensor(zs[:, tt, zt * 512:(zt + 1) * 512],
                                            pt[:], sigz[:], OP.mult)
            # dt block, column-major [16, t]; softplus(dtraw + dtb) fused
            for t2 in range(2):
                pt = ps_main.tile([128, 512], F32, tag="mm")
                for kt in range(8):
                    nc.tensor.matmul(
                        pt[:HL, :], wdt[:, kt, :], xT[:, kt, t2 * 512:(t2 + 1) * 512],
                        start=(kt == 0), stop=(kt == 7))
                dslc = dtT[:, t2 * 512:(t2 + 1) * 512]
                nc.scalar.activation(dslc, pt[:HL, :], AF.Exp, bias=dtb[:])
                nc.scalar.activation(dslc, dslc, AF.Ln, bias=1.0)

            # ---- conv + silu + transpose to token-major --------------------
            for pt_i in range(CONVT):
                acc = work.tile([128, L], F32, tag="convacc")
                nc.gpsimd.tensor_scalar_mul(acc[:], xbcT[:, pt_i, 0:L], convw[:, pt_i, 0:1])
                for k in range(1, 4):
                    nc.vector.scalar_tensor_tensor(
                        acc[:], xbcT[:, pt_i, k:k + L], convw[:, pt_i, k:k + 1], acc[:],
                        op0=OP.mult, op1=OP.add)
                nc.gpsimd.tensor_scalar_add(acc[:], acc[:], convb[:, pt_i:pt_i + 1])
                sigc = work.tile([128, L], BF, tag="sigc", name="sigc")
                nc.scalar.activation(sigc[:], acc[:], AF.Sigmoid)
                if pt_i < 8:
                    xs_scr = work.tile([128, L], BF, tag="xs_scr")
                    nc.vector.tensor_tensor(xs_scr[:], acc[:], sigc[:], OP.mult)
                    # transpose [hp-chunk, t] -> xsTok[t, tt, hp-chunk]
                    for tg in range(2):
                        ptr = ps_tr.tile([128, 512], BF, tag="trb")
                        for i in range(4):
                            tt = tg * 4 + i
                            nc.tensor.transpose(
                                ptr[:, i * 128:(i + 1) * 128],
                                xs_scr[:, tt * 128:(tt + 1) * 128], idb[:])
                        for i in range(4):
                            tt = tg * 4 + i
                            nc.scalar.copy(
                                xsTok[:, tt, pt_i * 128:(pt_i + 1) * 128],
                                ptr[:, i * 128:(i + 1) * 128])
                else:
                    nc.vector.tensor_tensor(BCt[:], acc[:], sigc[:], OP.mult)

            # B token-major [t, c, n] for the state-update matmuls
            Ct = res.tile([64, L], BF)
            nc.sync.dma_start(Ct[:], BCt[64:128, :])
            Btok = res.tile([128, NCH, 64], BF)
            for tg in range(2):
                ptr = ps_tr.tile([128, 512], BF, tag="trb")
                for i in range(4):
                    c = tg * 4 + i
                    nc.tensor.transpose(
                        ptr[:, i * 128:i * 128 + 64],
                        BCt[0:64, c * 128:(c + 1) * 128], idb[0:64, 0:64])
                for i in range(4):
                    c = tg * 4 + i
                    nc.scalar.copy(Btok[:, c, :], ptr[:, i * 128:i * 128 + 64])

            # ---- decay precompute ------------------------------------------
            nc.scalar.activation(cumlog2[:], dtT[:], AF.Ln)
            nc.vector.tensor_scalar_mul(logdA[:], dtT[:], negA[:])
            for c in range(NCH):
                sl = slice(c * Q, (c + 1) * Q)
                nc.vector.tensor_tensor_scan(
                    cum[:, sl], ones16[:], logdA[:, sl], 0.0, OP.mult, OP.add)
            nc.vector.tensor_tensor(cumlog2[:], cum[:], cumlog2[:], OP.subtract)
            nc.scalar.activation(expcum_cm[:], cum[:], AF.Exp)
            nc.sync.dma_start(cum_dram[:, :], cum[:])
            nc.sync.dma_start(ecum_dram[:, :], expcum_cm[:])

            for c in range(NCH):
                sl = slice(c * Q, (c + 1) * Q)
                dscm = work.tile([HL, Q], F32, tag="dscm")
                nc.scalar.activation(dscm[:], cumlog2[:, sl], AF.Exp,
                                     bias=cum[:, c * Q + Q - 1:c * Q + Q], scale=-1.0)
                ptr = ps_main.tile([128, 512], F32, tag="mm")
                nc.tensor.transpose(ptr[:, 0:HL], cum[:, sl], idf[0:HL, 0:HL])
                nc.tensor.transpose(ptr[:, 16:16 + HL], cumlog2[:, sl], idf[0:HL, 0:HL])
                nc.tensor.transpose(ptr[:, 32:32 + HL], dscm[:], idf[0:HL, 0:HL])
                nc.scalar.activation(expcumT[:, c, :], ptr[:, 0:HL], AF.Exp)
                nc.scalar.mul(negclT[:, c, :], ptr[:, 16:16 + HL], -1.0)
                nc.scalar.copy(dstateT[:, c, :], ptr[:, 32:32 + HL])
            # per-chunk per-head full-chunk decay, broadcast down 64 partitions
            lam_dram = nc.dram_tensor("lam_dram", [NCH, HL], F32)
            nc.sync.dma_start(lam_dram[:, :], expcumT[127:128, :, :])
            nc.sync.dma_start(
                lam.rearrange("p c h -> p (c h)"),
                bass.AP(tensor=lam_dram, offset=0, ap=[[0, 64], [1, NCH * HL]]))

            # ---- chunked scan ----------------------------------------------
            for c in range(NCH):
                sl = slice(c * Q, (c + 1) * Q)
                # broadcast cum rows across partitions (decay-matrix argument)
                cumrow = work1.tile([128, HL, Q], F32, tag="cumrow")
                for h in range(HL):
                    nc.sync.dma_start(
                        cumrow[:, h:h + 1, :],
                        bass.AP(tensor=cum_dram, offset=h * L + c * Q,
                                ap=[[0, 128], [0, 1], [1, Q]]))
                ecrow = work1.tile([64, HL, Q], BF, tag="ecrow")
                for h in range(HL):
                    nc.sync.dma_start(
                        ecrow[:, h:h + 1, :],
                        bass.AP(tensor=ecum_dram, offset=h * L + c * Q,
                                ap=[[0, 64], [0, 1], [1, Q]]))
                # arg[s, h, t] = cum[t] - BIG*(t<s)
                nc.vector.tensor_tensor(
                    cumrow[:], cumrow[:],
                    bass.AP(tensor=lowbig.tensor, offset=lowbig.offset,
                            ap=[lowbig.ap[0], [0, HL], lowbig.ap[1]]),
                    OP.subtract)
                # expMT[s, h, t] = exp(cum[t] - cumlog2[s] - BIG*(t<s)) -> WT
                WT = work1.tile([128, HL, Q], BF, tag="WT")
                for h in range(HL):
                    nc.scalar.activation(WT[:, h, :], cumrow[:, h, :], AF.Exp,
                                         bias=negclT[:, c, h:h + 1])
                # G^T[s, t] = B_s . C_t (shared across heads)
                pg = ps_g.tile([128, 128], F32, tag="g")
                nc.tensor.matmul(pg[:], BCt[0:64, sl], Ct[:, sl],
                                 start=True, stop=True)
                # W^T[s, h, t] = G^T * expMT + D*I
                nc.vector.tensor_tensor(
                    WT[:],
                    bass.AP(tensor=pg.tensor, offset=pg.offset,
                            ap=[pg.ap[0], [0, HL], pg.ap[1]]),
                    WT[:], OP.mult)
                nc.vector.tensor_tensor(WT[:], WT[:], diag[:], OP.add)
                # Ctilde[n, h, t] = C^T * exp(cum[t])  (in place into ecrow)
                Ctil = ecrow
                nc.vector.tensor_tensor(
                    Ctil[:],
                    bass.AP(tensor=Ct.tensor, offset=Ct[:, sl].offset,
                            ap=[Ct.ap[0], [0, HL], [1, Q]]),
                    ecrow[:], OP.mult)
                # Y matmuls -> psum_y[t, h, p], half the heads at a time
                for hh in range(2):
                    py = ps_y.tile([128, 8, 64], F32, tag="y")
                    for i in range(8):
                        h = hh * 8 + i
                        nc.tensor.matmul(py[:, i, :], WT[:, h, :],
                                         xsTok[:, c, h * 64:(h + 1) * 64],
                                         start=True, stop=(c == 0))
                        if c > 0:
                            nc.tensor.matmul(py[:, i, :], Ctil[:, h, :],
                                             Snp[:, h, :], start=False, stop=True)
                    # gating eviction: gb = psum_y * silu(z)
                    nc.vector.tensor_tensor(
                        gb[:, c, hh * 512:(hh + 1) * 512],
                        py[:, :, :].rearrange("p h q -> p (h q)"),
                        zs[:, c, hh * 512:(hh + 1) * 512], OP.mult)
                sqscr = res.tile([128, 1024], BF, tag="bigT", name="sqscr")
                nc.scalar.activation(sqscr[:], gb[:, c, :], AF.Square,
                                     accum_out=ss_sb[:, c:c + 1])
                # state update
                if c < NCH - 1:
                    Btil = work1.tile([128, HL, 64], BF, tag="Btil")
                    nc.vector.tensor_tensor(
                        Btil[:],
                        bass.AP(tensor=Btok.tensor, offset=Btok[:, c, :].offset,
                                ap=[Btok.ap[0], [0, HL], [1, 64]]),
                        bass.AP(tensor=dstateT.tensor,
                                offset=dstateT[:, c, :].offset,
                                ap=[dstateT.ap[0], [1, HL], [0, 64]]),
                        OP.mult)
                    pu = ps_u.tile([64, HL, 64], F32, tag="u")
                    for h in range(HL):
                        nc.tensor.matmul(
                            pu[:, h, :], Btil[:, h, :],
                            xsTok[:, c, h * 64:(h + 1) * 64],
                            start=True, stop=True)
                    for h in range(HL):
                        if c == 0:
                            nc.vector.tensor_copy(Snp[:, h, :], pu[:, h, :])
                        else:
                            nc.vector.scalar_tensor_tensor(
                                Snp[:, h, :], Snp[:, h, :], lam[:, c, h:h + 1],
                                pu[:, h, :], op0=OP.mult, op1=OP.add)

            # ---- transpose gb to column-major ------------------------------
            gbT = res.tile([128, 8, L], BF, tag="bigT")
            for tt in range(8):
                for pg_i in range(2):
                    ptr = ps_tr.tile([128, 512], BF, tag="trb")
                    for i in range(4):
                        hp = pg_i * 4 + i
                        nc.tensor.transpose(
                            ptr[:, i * 128:(i + 1) * 128],
                            gb[:, tt, hp * 128:(hp + 1) * 128], idb[:])
                    for i in range(4):
                        hp = pg_i * 4 + i
                        nc.scalar.copy(gbT[:, hp, tt * 128:(tt + 1) * 128],
                                       ptr[:, i * 128:(i + 1) * 128])

            # ---- out_proj partial: P = (g*normw) @ Wout --------------------
            for tt in range(8):
                for nt in range(2):
                    pt = ps_main.tile([128, 512], F32, tag="mm")
                    for kt in range(8):
                        nc.tensor.matmul(
                            pt[:], gbT[:, kt, tt * 128:(tt + 1) * 128],
                            wout[:, kt, nt * 512:(nt + 1) * 512],
                            start=(kt == 0), stop=(kt == 7))
                    po = work.tile([128, 512], F32, tag="Pout")
                    nc.scalar.copy(po[:], pt[:])
                    nc.sync.dma_start(P_d[tt, :, nt * 512:(nt + 1) * 512], po[:])
            nc.sync.dma_start(ss_d.rearrange("c p -> p c"), ss_sb[:])

    nc.compile()
    return nc


# ----------------------------------------------------------------------------
# Launch 2: out = concat([fwd, bwd]) @ Wo  (token-parallel)
# ----------------------------------------------------------------------------
def build_wo():
    nc = bacc.Bacc()
    rT_d = nc.dram_tensor("rT", [16, 128, 256], BF, kind="ExternalInput")
    wo_d = nc.dram_tensor("wo", [16, 128, 1024], BF, kind="ExternalInput")
    o_d = nc.dram_tensor("o", [2, 128, 1024], F32, kind="ExternalOutput")
    with tile.TileContext(nc) as tc:
        with (
            tc.tile_pool(name="res", bufs=1) as res,
            tc.tile_pool(name="work", bufs=3) as work,
            tc.tile_pool(name="ps", bufs=4, space="PSUM") as ps,
        ):
            rT = res.tile([128, 16, 256], BF)
            nc.sync.dma_start(rT[:], rT_d.rearrange("k p t -> p k t"))
            wo = res.tile([128, 16, 1024], BF)
            nc.sync.dma_start(wo[:], wo_d.rearrange("k p t -> p k t"))
            for tt in range(2):
                for nt in range(2):
                    pt = ps.tile([128, 512], F32, tag="mm")
                    for kt in range(16):
                        nc.tensor.matmul(
                            pt[:], rT[:, kt, tt * 128:(tt + 1) * 128],
                            wo[:, kt, nt * 512:(nt + 1) * 512],
                            start=(kt == 0), stop=(kt == 15))
                    po = work.tile([128, 512], F32, tag="o")
                    nc.scalar.copy(po[:], pt[:])
                    nc.sync.dma_start(o_d[tt, :, nt * 512:(nt + 1) * 512], po[:])
    nc.compile()
    return nc


# ----------------------------------------------------------------------------
# Host orchestration
# ----------------------------------------------------------------------------
_cache = {}


def _get_core_nc():
    if "core" not in _cache:
        _cache["core"] = build_core()
    return _cache["core"]


def _get_wo_nc():
    if "wo" not in _cache:
        _cache["wo"] = build_wo()
    return _cache["wo"]


def _prep_core_inputs(x_seq, Win, convw, convb, dtb, Alog, Dsk, normw, Wout, h):
    """Host-side shard/cast for one (dir, batch, half) core."""
    xs_cols = np.arange(2048 + h * 1024, 2048 + (h + 1) * 1024)
    bc_cols = np.arange(4096, 4224)
    z_cols = np.arange(h * 1024, (h + 1) * 1024)
    dt_cols = np.arange(4224 + h * HL, 4224 + (h + 1) * HL)
    heads = slice(h * HL, (h + 1) * HL)
    ch = slice(h * 1024, (h + 1) * 1024)

    wxbc = Win[:, np.concatenate([xs_cols, bc_cols])]          # [1024, 1152]
    wz = Win[:, z_cols]                                        # [1024, 1024]
    wdt = Win[:, dt_cols]                                      # [1024, 16]
    wout = normw[ch, None] * Wout[ch]                          # [1024, 1024]

    conv_idx = np.concatenate([np.arange(h * 1024, (h + 1) * 1024),
                               np.arange(2048, 2176)])
    cw = convw[:, 0, conv_idx].T.copy()                        # [1152, 4]
    cb = convb[conv_idx]                                       # [1152]

    D = Dsk[heads]
    diag = (np.eye(128, dtype=np.float32)[:, None, :]
            * D[None, :, None]).astype(BF16)                   # [128, 16, 128]
    lowbig = (BIG * (np.arange(128)[None, :] < np.arange(128)[:, None])
              ).astype(np.float32)                             # [s, t]

    return {
        "xT": np.ascontiguousarray(x_seq.T.reshape(8, 128, L).astype(BF16)),
        "wxbc": np.ascontiguousarray(wxbc.reshape(8, 128, 1152).astype(BF16)),
        "wz": np.ascontiguousarray(wz.reshape(8, 128, 1024).astype(BF16)),
        "wdt": np.ascontiguousarray(wdt.reshape(8, 128, HL).astype(BF16)),
        "wout": np.ascontiguousarray(wout.reshape(8, 128, 1024).astype(BF16)),
        "convw": np.ascontiguousarray(cw.reshape(CONVT, 128, 4)),
        "convb": np.ascontiguousarray(cb.reshape(CONVT, 128)),
        "dtb": np.ascontiguousarray(dtb[heads]).astype(np.float32),
        "negA": (-np.exp(Alog[heads])).astype(np.float32),
        "diag": diag,
        "lowbig": lowbig,
        "idf": np.eye(128, dtype=np.float32),
        "idb": np.eye(128, dtype=np.float32).astype(BF16),
    }


def kernel(**inputs):
    x = np.asarray(inputs["x"], dtype=np.float32)              # [2, L, DM]

    dir_params = {}
    for d, pre in (("f", "f"), ("b", "b")):
        dir_params[d] = tuple(np.asarray(inputs[pre + k], dtype=np.float32)
                              for k in ("Win", "convw", "convb", "dtb",
                                        "Alog", "D", "normw", "Wout"))

    # core order: (dir, batch, half)
    core_keys = [(d, b, h) for d in ("f", "b") for b in range(2) for h in range(2)]
    in_maps = []
    for (d, b, h) in core_keys:
        x_seq = x[b] if d == "f" else x[b, ::-1]
        in_maps.append(_prep_core_inputs(x_seq, *dir_params[d], h))

    nc = _get_core_nc()
    res = bass_utils.run_bass_kernel_spmd(nc, in_maps, core_ids=list(range(8)))

    # host combine: rsqrt over both halves, flip bwd, concat
    r = {}
    for ci, (d, b, h) in enumerate(core_keys):
        P = res.results[ci]["P"].reshape(L, 1024)
        ss = res.results[ci]["ss"].reshape(L)
        key = (d, b)
        if key not in r:
            r[key] = [P, ss]
        else:
            r[key][0] = r[key][0] + P
            r[key][1] = r[key][1] + ss
    R = np.zeros((2, L, 2048), np.float32)
    for (d, b), (Psum, sssum) in r.items():
        rs = 1.0 / np.sqrt(sssum / 2048.0 + 1e-5)
        out_db = rs[:, None] * Psum
        if d == "b":
            out_db = out_db[::-1]
        R[b, :, (0 if d == "f" else 1024):(1024 if d == "f" else 2048)] = out_db

    # launch 2: token-parallel Wo
    Wo = np.asarray(inputs["Wo"], dtype=np.float32)
    bo = np.asarray(inputs["bo"], dtype=np.float32)
    wo_bf = np.ascontiguousarray(Wo.reshape(16, 128, 1024).astype(BF16))
    Rflat = R.reshape(2048, 2048)
    in_maps2 = []
    for c in range(8):
        sl = Rflat[c * 256:(c + 1) * 256]                      # [256, 2048]
        in_maps2.append({
            "rT": np.ascontiguousarray(sl.T.reshape(16, 128, 256).astype(BF16)),
            "wo": wo_bf,
        })
    nc2 = _get_wo_nc()
    res2 = bass_utils.run_bass_kernel_spmd(nc2, in_maps2, core_ids=list(range(8)))
    out = np.concatenate([res2.results[c]["o"].reshape(256, 1024)
                          for c in range(8)], 0)
    out = out.reshape(2, L, DM) + bo[None, None, :]
    return out.astype(np.float32)
